# revision 1
# baseline (speedup 1.0000x reference)
"""AR(1) model kernel for Trainium2, 8-core data parallel.

Computes out[b,t,n,0] = x[b,t-1,n,0]*w + bias for t>=1, out[b,0,n,0] = 0,
for x of shape (64, 288, 2000, 1), w = weights[0,0,0], bias scalar.

Sharding: pure data parallel on batch — 8 batches per core. Replicate w/bias.
Per core the work is a shifted scaled copy: for each local batch b, the
574,000-float block x[b, 0:287, :] maps contiguously to out[b, 1:288, :].
574000 = 112*5125, so one [112, 5125] DMA covers a whole batch each way.
"""

import numpy as np

import concourse.bacc as bacc
import concourse.mybir as mybir
import concourse.tile as tile
from concourse import bass_utils

B, T, N = 64, 288, 2000
NCORES = 8
BL = B // NCORES          # 8 local batches per core
TN = T * N                # 576000 floats per batch
BODY = (T - 1) * N        # 574000 floats shifted per batch
TOT = BL * TN             # 4608000 floats per core

PART = 112                # 574000 = 112 * 5125
FREE = BODY // PART       # 5125

_nc_cache = {}


def _build_nc():
    nc = bacc.Bacc(
        "TRN2", target_bir_lowering=False, debug=False, num_devices=NCORES
    )
    f32 = mybir.dt.float32
    x = nc.dram_tensor("x", [TOT], f32, kind="ExternalInput").ap()
    wb = nc.dram_tensor("wb", [1, 2], f32, kind="ExternalInput").ap()
    out = nc.dram_tensor("out", [TOT], f32, kind="ExternalOutput").ap()

    with tile.TileContext(nc) as tc:
        with (
            tc.tile_pool(name="consts", bufs=1) as consts,
            tc.tile_pool(name="data", bufs=4) as data,
        ):
            # w and bias broadcast to all partitions: [128, 2] tile.
            wbt = consts.tile([128, 2], f32)
            nc.sync.dma_start(wbt[:], wb.to_broadcast((128, 2)))

            # Zero rows t=0 of every local batch: one strided [BL, N] store.
            zt = consts.tile([BL, N], f32)
            nc.vector.memset(zt[:], 0.0)
            out2d = out.rearrange("(b q) -> b q", b=BL)
            nc.sync.dma_start(out2d[:, 0:N], zt[:])

            for b in range(BL):
                xo = b * TN
                oo = b * TN + N
                t0 = data.tile([PART, FREE], f32)
                nc.sync.dma_start(
                    t0[:], x[xo : xo + BODY].rearrange("(p f) -> p f", p=PART)
                )
                nc.vector.tensor_scalar(
                    t0[:],
                    t0[:],
                    wbt[0:PART, 0:1],
                    wbt[0:PART, 1:2],
                    mybir.AluOpType.mult,
                    mybir.AluOpType.add,
                )
                nc.sync.dma_start(
                    out[oo : oo + BODY].rearrange("(p f) -> p f", p=PART), t0[:]
                )

    nc.compile()
    return nc


def _get_nc():
    if "nc" not in _nc_cache:
        _nc_cache["nc"] = _build_nc()
    return _nc_cache["nc"]


def kernel(x, weights, bias, _trace=False):
    x = np.ascontiguousarray(np.asarray(x, dtype=np.float32)).reshape(B, TN)
    wb = np.array(
        [[np.asarray(weights).reshape(-1)[0], np.asarray(bias).reshape(-1)[0]]],
        dtype=np.float32,
    )
    in_maps = [
        {"x": x[c * BL : (c + 1) * BL].reshape(-1), "wb": wb} for c in range(NCORES)
    ]
    nc = _get_nc()
    res = bass_utils.run_bass_kernel_spmd(
        nc, in_maps, core_ids=list(range(NCORES)), trace=_trace
    )
    out = np.concatenate(
        [res.results[c]["out"].reshape(BL, T, N, 1) for c in range(NCORES)], axis=0
    )
    if _trace:
        return out, res
    return out


# revision 2
# speedup vs baseline: 1.0487x; 1.0487x over previous
"""AR(1) model kernel for Trainium2, 8-core data parallel.

Computes out[b,t,n,0] = x[b,t-1,n,0]*w + bias for t>=1, out[b,0,n,0] = 0,
for x of shape (64, 288, 2000, 1), w = weights[0,0,0], bias scalar.

Sharding: pure data parallel on batch — 8 batches per core. Replicate w/bias.
Per core the work is a shifted scaled copy: for each local batch b, the
574,000-float block x[b, 0:287, :] maps contiguously to out[b, 1:288, :].
574000 = 112*5125, so one [112, 5125] DMA covers a whole batch each way.
"""

import numpy as np

import concourse.bacc as bacc
import concourse.mybir as mybir
import concourse.tile as tile
from concourse import bass_utils

B, T, N = 64, 288, 2000
NCORES = 8
BL = B // NCORES          # 8 local batches per core
TN = T * N                # 576000 floats per batch
BODY = (T - 1) * N        # 574000 floats shifted per batch
TOT = BL * TN             # 4608000 floats per core

PART = 112                # 574000 = 112 * 5125
FREE = BODY // PART       # 5125

_nc_cache = {}


def _build_nc():
    nc = bacc.Bacc(
        "TRN2", target_bir_lowering=False, debug=False, num_devices=NCORES
    )
    f32 = mybir.dt.float32
    x = nc.dram_tensor("x", [TOT], f32, kind="ExternalInput").ap()
    wb = nc.dram_tensor("wb", [1, 2], f32, kind="ExternalInput").ap()
    out = nc.dram_tensor("out", [TOT], f32, kind="ExternalOutput").ap()

    with tile.TileContext(nc) as tc:
        with (
            tc.tile_pool(name="consts", bufs=1) as consts,
            tc.tile_pool(name="data", bufs=6) as data,
        ):
            # w and bias broadcast to all partitions: [128, 2] tile.
            wbt = consts.tile([128, 2], f32)
            nc.sync.dma_start(wbt[:], wb.to_broadcast((128, 2)))

            # Zero rows t=0 of every local batch: one strided [BL, N] store.
            # Issued on the ACT (scalar) HWDGE ring with the other stores.
            zt = consts.tile([BL, N], f32)
            nc.vector.memset(zt[:], 0.0)
            out2d = out.rearrange("(b q) -> b q", b=BL)
            nc.scalar.dma_start(out2d[:, 0:N], zt[:])

            # Loads issue on the SP (sync) HWDGE ring, stores on the ACT
            # (scalar) ring, so a store waiting on compute never head-of-line
            # blocks the next batch's load.
            for b in range(BL):
                xo = b * TN
                oo = b * TN + N
                t0 = data.tile([PART, FREE], f32)
                nc.sync.dma_start(
                    t0[:], x[xo : xo + BODY].rearrange("(p f) -> p f", p=PART)
                )
                nc.vector.tensor_scalar(
                    t0[:],
                    t0[:],
                    wbt[0:PART, 0:1],
                    wbt[0:PART, 1:2],
                    mybir.AluOpType.mult,
                    mybir.AluOpType.add,
                )
                nc.scalar.dma_start(
                    out[oo : oo + BODY].rearrange("(p f) -> p f", p=PART), t0[:]
                )

    nc.compile()
    return nc


def _get_nc():
    if "nc" not in _nc_cache:
        _nc_cache["nc"] = _build_nc()
    return _nc_cache["nc"]


def kernel(x, weights, bias, _trace=False):
    x = np.ascontiguousarray(np.asarray(x, dtype=np.float32)).reshape(B, TN)
    wb = np.array(
        [[np.asarray(weights).reshape(-1)[0], np.asarray(bias).reshape(-1)[0]]],
        dtype=np.float32,
    )
    in_maps = [
        {"x": x[c * BL : (c + 1) * BL].reshape(-1), "wb": wb} for c in range(NCORES)
    ]
    nc = _get_nc()
    res = bass_utils.run_bass_kernel_spmd(
        nc, in_maps, core_ids=list(range(NCORES)), trace=_trace
    )
    out = np.concatenate(
        [res.results[c]["out"].reshape(BL, T, N, 1) for c in range(NCORES)], axis=0
    )
    if _trace:
        return out, res
    return out


# revision 3
# speedup vs baseline: 1.1616x; 1.1077x over previous
"""AR(1) model kernel for Trainium2, 8-core data parallel.

Computes out[b,t,n,0] = x[b,t-1,n,0]*w + bias for t>=1, out[b,0,n,0] = 0,
for x of shape (64, 288, 2000, 1), w = weights[0,0,0], bias scalar.

Sharding: pure data parallel on batch — 8 batches per core. Replicate w/bias.
Per core the work is a shifted scaled copy: for each local batch b, the
574,000-float block x[b, 0:287, :] maps contiguously to out[b, 1:288, :].
574000 = 112*5125, so one [112, 5125] DMA covers a whole batch each way.
"""

import numpy as np

import concourse.bacc as bacc
import concourse.mybir as mybir
import concourse.tile as tile
from concourse import bass_utils

B, T, N = 64, 288, 2000
NCORES = 8
BL = B // NCORES          # 8 local batches per core
TN = T * N                # 576000 floats per batch
BODY = (T - 1) * N        # 574000 floats shifted per batch
TOT = BL * TN             # 4608000 floats per core

PART = 112                # 574000 = 112 * 5125
FREE = BODY // PART       # 5125

_nc_cache = {}


def _build_nc():
    nc = bacc.Bacc(
        "TRN2", target_bir_lowering=False, debug=False, num_devices=NCORES
    )
    f32 = mybir.dt.float32
    x = nc.dram_tensor("x", [TOT], f32, kind="ExternalInput").ap()
    wb = nc.dram_tensor("wb", [1, 2], f32, kind="ExternalInput").ap()
    out = nc.dram_tensor("out", [TOT], f32, kind="ExternalOutput").ap()

    with tile.TileContext(nc) as tc:
        with (
            tc.tile_pool(name="consts", bufs=1) as consts,
            tc.tile_pool(name="data", bufs=6) as data,
        ):
            # Tiny setup transfers go through SWDGE (gpsimd) so the two
            # HWDGE rings are reserved for the big streaming transfers:
            # loads on the SP (sync) ring, stores on the ACT (scalar) ring.
            # That way a store waiting on compute never head-of-line blocks
            # the next batch's load.
            wbt = consts.tile([128, 2], f32)
            nc.gpsimd.dma_start(wbt[:], wb.to_broadcast((128, 2)))

            # Zero rows t=0 of every local batch: one strided [BL, N] store.
            zt = consts.tile([BL, N], f32)
            nc.gpsimd.memset(zt[:], 0.0)
            out2d = out.rearrange("(b q) -> b q", b=BL)
            nc.gpsimd.dma_start(out2d[:, 0:N], zt[:])

            def piece(xo, oo, part, free, col0, cols):
                """load/scale/store one [part, cols] slice of a batch."""
                t0 = data.tile([part, cols], f32, tag="t0")
                src = x[xo : xo + part * free].rearrange("(p f) -> p f", p=part)
                dst = out[oo : oo + part * free].rearrange("(p f) -> p f", p=part)
                nc.sync.dma_start(t0[:], src[:, col0 : col0 + cols])
                nc.vector.tensor_scalar(
                    t0[:],
                    t0[:],
                    wbt[0:part, 0:1],
                    wbt[0:part, 1:2],
                    mybir.AluOpType.mult,
                    mybir.AluOpType.add,
                )
                nc.scalar.dma_start(dst[:, col0 : col0 + cols], t0[:])

            # First and last batches are split into half-chunks: the fill
            # ramp (nothing to store yet) and the drain ramp (nothing left
            # to load) each waste half the HBM duplex bandwidth, so shorter
            # lone transfers at the edges shrink that window.
            H1, H2 = 2565, 2560  # 5125 split
            for b in range(BL):
                xo = b * TN
                oo = b * TN + N
                if b == 0 or b == BL - 1:
                    piece(xo, oo, PART, FREE, 0, H1)
                    piece(xo, oo, PART, FREE, H1, H2)
                else:
                    piece(xo, oo, PART, FREE, 0, FREE)

    nc.compile()
    return nc


def _get_nc():
    if "nc" not in _nc_cache:
        _nc_cache["nc"] = _build_nc()
    return _nc_cache["nc"]


def kernel(x, weights, bias, _trace=False):
    x = np.ascontiguousarray(np.asarray(x, dtype=np.float32)).reshape(B, TN)
    wb = np.array(
        [[np.asarray(weights).reshape(-1)[0], np.asarray(bias).reshape(-1)[0]]],
        dtype=np.float32,
    )
    in_maps = [
        {"x": x[c * BL : (c + 1) * BL].reshape(-1), "wb": wb} for c in range(NCORES)
    ]
    nc = _get_nc()
    res = bass_utils.run_bass_kernel_spmd(
        nc, in_maps, core_ids=list(range(NCORES)), trace=_trace
    )
    out = np.concatenate(
        [res.results[c]["out"].reshape(BL, T, N, 1) for c in range(NCORES)], axis=0
    )
    if _trace:
        return out, res
    return out
